# revision 4
# baseline (speedup 1.0000x reference)
"""Dense dot-product attention (B=16, S=2048, D=128, fp32) on 8 TRN2 NeuronCores.

Sharding: data-parallel over batch — each of the 8 cores processes 2 full
batches independently (no collectives).

Per-core algorithm (per batch b, D=128, S=2048), all matmul operands fp16
(sim'd end-to-end rel err ~5e-4 vs the 2e-2 gate; fp16 matmul = 1 PE
cycle/row, same as f32r/bf16, but transposes are 1.0 vs 1.5 cycles/row and
all SBUF traffic halves):

  - Load Q, K, V naturally ([s, d] tiles), cast to fp16 on DVE.
  - PE-transpose Q and K into [d, s] layout (QT, KT) — matmuls contract
    over the partition dim, so the d-contraction of Q@K^T needs d on
    partitions.
  - Queries processed in 4 chunks of 512. Per chunk, k-tiles processed in
    8 PAIRS (2 x 128 keys). Per pair:
      S^T[k, q]  = 2 matmuls into one [128, 2, 512] PSUM pair tile
      P^T[k, q]  = exp(S^T / sqrt(D)) — ONE ScalarE activation per pair
                   ([128, 2, 512], 1024 elem) halving the per-instruction
                   ScalarE bubble (~185ns each); fp16 SBUF output
      O^T[d, q] += 2 matmuls(lhsT=V_tile fp16, rhs=P^T slot)  (PSUM acc)
      Zacc      += P^T pair (GPSIMD/Pool elementwise accumulate — the Pool
                   engine is otherwise idle; this removes the ones-matmul
                   row-sum from the PE entirely, -25% PE work)
  - Chunk end: Z[q] = ones[128,8]^T @ Zacc slots (2 tiny accumulating
    matmuls, [8, 512] PSUM out: matmul cost scales with out FREE size, so
    the 8-partition replication makes the later Z transpose ~16x cheaper);
    O^T evacuated to SBUF as fp16.
  - Deferred epilogue (runs inside the next chunk's k-loop so the PE never
    stalls): transpose Z columns ([8,128] -> [128,8], 8 moving rows each),
    reciprocal on DVE, PE-transpose O^T back to [q, d] fp16 tiles,
    normalize during PSUM evacuation (tensor_scalar mul by 1/Z[q]), DMA out.

Engine budget per core (measured/predicted): ScalarE exp ~68us (pacer),
PE ~57us, Pool ~59us, DVE ~40us.
"""

import math
import sys
from contextlib import ExitStack

try:
    import concourse.bass  # noqa: F401
except ImportError:
    for _p in ("/opt/trn_rl_repo", "/root/.axon_site/_ro/trn_rl_repo"):
        if _p not in sys.path:
            sys.path.insert(0, _p)

import numpy as np

import concourse.bass as bass
import concourse.mybir as mybir
import concourse.tile as tile
from concourse import bacc
from concourse.bass_utils import run_bass_kernel_spmd
from concourse.masks import make_identity

B, S, D = 16, 2048, 128
N_CORES = 8
B_LOC = B // N_CORES  # batches per core
P = 128
N_KT = S // P          # k tiles per batch (16)
N_KP = N_KT // 2       # k-tile PAIRS per batch (8)
QCHUNK = 512           # queries per accumulation pass
N_QC = S // QCHUNK     # q chunks per batch (4)
NQT = QCHUNK // P      # output q tiles per chunk (4)
SOFTMAX_SCALE = 1.0 / math.sqrt(D)

F32 = mybir.dt.float32
F32R = mybir.dt.float32r
F16 = mybir.dt.float16


def build_attention_nc() -> bass.Bass:
    nc = bacc.Bacc()
    q_in = nc.declare_dram_parameter("query", [B_LOC, S, D], F32, isOutput=False)
    k_in = nc.declare_dram_parameter("key", [B_LOC, S, D], F32, isOutput=False)
    v_in = nc.declare_dram_parameter("value", [B_LOC, S, D], F32, isOutput=False)
    o_out = nc.declare_dram_parameter("out", [B_LOC, S, D], F32, isOutput=True)

    with tile.TileContext(nc) as tc, ExitStack() as ctx:
        const = ctx.enter_context(tc.tile_pool(name="const", bufs=1))
        io = ctx.enter_context(tc.tile_pool(name="io", bufs=2))
        tr = ctx.enter_context(tc.tile_pool(name="tr", bufs=2))
        pexp = ctx.enter_context(tc.tile_pool(name="pexp", bufs=4))
        norm = ctx.enter_context(tc.tile_pool(name="norm", bufs=2))
        # PSUM: sc pairs 2x2 banks + outT 1 + z ring 2 + tp 1 = 8 banks
        ps_s = ctx.enter_context(tc.tile_pool(name="ps_s", bufs=2, space="PSUM"))
        ps_acc = ctx.enter_context(tc.tile_pool(name="ps_acc", bufs=1, space="PSUM"))

        identity = const.tile([P, P], F32)
        make_identity(nc, identity)
        identity_h = const.tile([P, P], F16)
        nc.vector.tensor_copy(identity_h[:], identity[:])
        ones8_f = const.tile([P, 8], F32)
        nc.gpsimd.memset(ones8_f[:], 1.0)
        ones8 = const.tile([P, 8], F32R)
        nc.vector.tensor_copy(ones8[:], ones8_f[:])

        pending_z = None
        pending_fin = None

        # ---- per-batch input prep, split into pipelinable steps ----
        def emit_v_half(v_nat, v_f16, b, h):
            sl = slice(h * (N_KT // 2), (h + 1) * (N_KT // 2))
            nc.sync.dma_start(
                v_nat[:, sl, :],
                v_in[b, h * (S // 2) : (h + 1) * (S // 2), :].rearrange(
                    "(t p) d -> p t d", p=P
                ),
            )
            nc.vector.tensor_copy(v_f16[:, sl, :], v_nat[:, sl, :])

        def emit_qk_load(src_in, b, j4, tagp):
            # DMA + fp16 cast only — queues on Sync/DVE, never blocks the PE
            nat = io.tile(
                [P, 4, D], F32, tag="qknat", name=f"nat_{tagp}_{b}_{j4}", bufs=8
            )
            nc.sync.dma_start(
                nat[:],
                src_in[b, j4 * 4 * P : (j4 + 1) * 4 * P, :].rearrange(
                    "(t p) d -> p t d", p=P
                ),
            )
            rnd = io.tile(
                [P, 4, D], F16, tag="qkrnd", name=f"rnd_{tagp}_{b}_{j4}", bufs=8
            )
            nc.vector.tensor_copy(rnd[:], nat[:])
            return rnd

        def emit_qk_transp(rnd, b, j4, dst):
            pst = ps_s.tile([P, 4, P], F16, tag="tp", name=f"pst_{b}_{j4}", bufs=1)
            for jj in range(4):
                nc.tensor.transpose(pst[:, jj, :], rnd[:, jj, :], identity_h[:])
            nc.vector.tensor_copy(
                dst[:, j4 * 4 * P : (j4 + 1) * 4 * P], pst[:]
            )

        def make_prep_steps(b):
            """Returns (qt, kt, v_f16, steps, n_defer)."""
            qt = tr.tile([P, S], F16, tag="qt", name=f"qt_{b}")
            kt = tr.tile([P, S], F16, tag="kt", name=f"kt_{b}")
            v_nat = io.tile([P, N_KT, D], F32, tag="vnat", name=f"vnat_{b}")
            v_f16 = io.tile([P, N_KT, D], F16, tag="vf16", name=f"vf16_{b}")

            def qk_split(src_in, j4, dst, tagp):
                box = {}

                def load():
                    box["rnd"] = emit_qk_load(src_in, b, j4, tagp)

                def transp():
                    emit_qk_transp(box["rnd"], b, j4, dst)

                return load, transp

            kp_ = [qk_split(k_in, j4, kt, "k") for j4 in range(N_KT // 4)]
            qp_ = [qk_split(q_in, j4, qt, "q") for j4 in range(N_KT // 4)]

            if b == 0:
                # chunk 0 needs kt groups just-in-time, qt group 0, V first.
                steps = [
                    kp_[0][0], qp_[0][0],       # loads k0, q0
                    kp_[0][1], qp_[0][1],       # transp k0, q0
                    lambda: emit_v_half(v_nat, v_f16, b, 0),
                    kp_[1][0], kp_[2][0], kp_[3][0],
                    lambda: emit_v_half(v_nat, v_f16, b, 1),
                    qp_[1][0], qp_[2][0], qp_[3][0],
                ]
                deferred = [
                    kp_[1][1], kp_[2][1], kp_[3][1],
                    qp_[1][1], qp_[2][1], qp_[3][1],
                ]
            else:
                # all emitted as deferred steps inside previous batch's loop
                steps = []
                deferred = [
                    kp_[0][0], kp_[1][0], kp_[2][0], kp_[3][0],
                    qp_[0][0],
                    lambda: emit_v_half(v_nat, v_f16, b, 0),
                    lambda: emit_v_half(v_nat, v_f16, b, 1),
                    kp_[0][1], kp_[1][1], kp_[2][1], kp_[3][1],
                    qp_[0][1],
                    qp_[1][0], qp_[2][0], qp_[3][0],
                    qp_[1][1], qp_[2][1], qp_[3][1],
                ]
            return qt, kt, v_f16, steps, deferred

        prep = {0: make_prep_steps(0)}
        deferred_steps: list = []

        for b in range(B_LOC):
            qt, kt, v_f16, steps, deferred = prep[b]
            for st in steps:
                st()
            deferred_steps.extend(deferred)
            steps.clear()

            if b + 1 < B_LOC:
                prep[b + 1] = make_prep_steps(b + 1)
                # next batch's prep runs inside this batch's k-loops
                deferred_steps.extend(prep[b + 1][3])
                deferred_steps.extend(prep[b + 1][4])
                prep[b + 1][3].clear()
                prep[b + 1][4].clear()

            def emit_sc_pair(q_lo, kp):
                sc = ps_s.tile(
                    [P, 2, QCHUNK], F32, tag=f"sc{kp % 2}", name=f"sc_{kp}", bufs=1
                )
                for i in range(2):
                    nc.tensor.matmul(
                        sc[:, i, :],
                        kt[:, (2 * kp + i) * P : (2 * kp + i + 1) * P],
                        qt[:, q_lo : q_lo + QCHUNK],
                        start=True,
                        stop=True,
                    )
                return sc

            def emit_epilogue_z(Zacc, z_sb):
                # Z[q] row-sums via 2 accumulating matmuls; [8, 512] out
                z_ps = ps_s.tile([8, QCHUNK], F32, tag="z", name="z_ps", bufs=2)
                for i in range(2):
                    nc.tensor.matmul(
                        z_ps[:],
                        ones8[:, :8],
                        Zacc[:, i, :],
                        start=(i == 0),
                        stop=(i == 1),
                    )
                nc.vector.tensor_copy(z_sb[:], z_ps[:])

            def emit_epilogue_fin(b, q_lo, o_un, z_sb):
                # transpose Z columns: [8, 128] -> [128, 8] (8 moving rows)
                zt_ps = ps_s.tile([P, NQT, 8], F32, tag="z", name="zt_ps", bufs=2)
                for j in range(NQT):
                    nc.tensor.transpose(
                        zt_ps[:, j, :],
                        z_sb[:8, j * P : (j + 1) * P],
                        identity[:8, :8],
                    )
                zr = norm.tile([P, NQT], F32, tag="zr")
                nc.vector.reciprocal(zr[:], zt_ps[:, :, 0])

                # transpose O^T back to [q, d]; normalize during evacuation
                pst = ps_s.tile([P, NQT, D], F16, tag="tp", name="pst_o", bufs=1)
                for j in range(NQT):
                    nc.tensor.transpose(
                        pst[:, j, :], o_un[:, j * P : (j + 1) * P], identity_h[:]
                    )
                out_sb = norm.tile([P, NQT, D], F32, tag="osb")
                for j in range(NQT):
                    nc.vector.tensor_scalar_mul(
                        out_sb[:, j, :], pst[:, j, :], zr[:, j : j + 1]
                    )
                nc.sync.dma_start(
                    o_out[b, q_lo : q_lo + QCHUNK, :].rearrange(
                        "(t p) d -> p t d", p=P
                    ),
                    out_sb[:],
                )

            for qc in range(N_QC):
                q_lo = qc * QCHUNK
                outT = ps_acc.tile([P, QCHUNK], F32, tag="outT", name="outT")
                Zacc = tr.tile(
                    [P, 2, QCHUNK], F32R, tag="zacc", name=f"zacc_{b}_{qc}"
                )

                sc = emit_sc_pair(q_lo, 0)
                for kp in range(N_KP):
                    pt = pexp.tile(
                        [P, 2, QCHUNK], F16, tag="pt", name=f"pt_{kp}", bufs=4
                    )
                    nc.scalar.activation(
                        pt[:],
                        sc[:],
                        mybir.ActivationFunctionType.Exp,
                        scale=SOFTMAX_SCALE,
                    )
                    # deferred prep must be emitted BEFORE the next score
                    # pair: Tile deps are emission-ordered, and sc(kp+1) may
                    # read kt columns written by a deferred transpose
                    if kp == 1 and pending_z is not None:
                        pending_z()
                        pending_z = None
                    elif kp == 2 and pending_fin is not None:
                        pending_fin()
                        pending_fin = None
                    elif deferred_steps and kp >= 1:
                        deferred_steps.pop(0)()
                    if kp + 1 < N_KP:
                        sc = emit_sc_pair(q_lo, kp + 1)
                    for i in range(2):
                        nc.tensor.matmul(
                            outT[:],
                            v_f16[:, 2 * kp + i, :],
                            pt[:, i, :],
                            start=(kp == 0 and i == 0),
                            stop=(kp == N_KP - 1 and i == 1),
                        )
                    # Pool accumulates the softmax denominator elementwise
                    if kp == 0:
                        nc.gpsimd.tensor_copy(Zacc[:], pt[:])
                    else:
                        nc.gpsimd.tensor_tensor(
                            Zacc[:], Zacc[:], pt[:], mybir.AluOpType.add
                        )

                # evacuate accumulator (frees the PSUM bank for next chunk)
                o_un = norm.tile([P, QCHUNK], F16, tag="o_un")
                nc.vector.tensor_copy(o_un[:], outT[:])
                z_sb = norm.tile([8, QCHUNK], F32, tag="z_sb")

                pending_z = (
                    lambda Zacc=Zacc, z_sb=z_sb: emit_epilogue_z(Zacc, z_sb)
                )
                pending_fin = (
                    lambda b=b, q_lo=q_lo, o_un=o_un, z_sb=z_sb: emit_epilogue_fin(
                        b, q_lo, o_un, z_sb
                    )
                )

        if pending_z is not None:
            pending_z()
        if pending_fin is not None:
            pending_fin()
        while deferred_steps:
            deferred_steps.pop(0)()

    nc.compile()
    return nc


_NC_CACHE: bass.Bass | None = None


def _get_nc() -> bass.Bass:
    global _NC_CACHE
    if _NC_CACHE is None:
        _NC_CACHE = build_attention_nc()
    return _NC_CACHE


def kernel(query: np.ndarray, key: np.ndarray, value: np.ndarray) -> np.ndarray:
    query = np.ascontiguousarray(np.asarray(query, dtype=np.float32))
    key = np.ascontiguousarray(np.asarray(key, dtype=np.float32))
    value = np.ascontiguousarray(np.asarray(value, dtype=np.float32))
    assert query.shape == (B, S, D), query.shape

    nc = _get_nc()
    core_ids = list(range(N_CORES))
    in_maps = [
        {
            "query": query[i * B_LOC : (i + 1) * B_LOC],
            "key": key[i * B_LOC : (i + 1) * B_LOC],
            "value": value[i * B_LOC : (i + 1) * B_LOC],
        }
        for i in range(N_CORES)
    ]
    res = run_bass_kernel_spmd(nc, in_maps, core_ids)
    out = np.concatenate([res.results[i]["out"] for i in range(N_CORES)], axis=0)
    return out


if __name__ == "__main__":
    rng = np.random.default_rng(0)
    q = rng.standard_normal((B, S, D)).astype(np.float32)
    k = rng.standard_normal((B, S, D)).astype(np.float32)
    v = rng.standard_normal((B, S, D)).astype(np.float32)
    o = kernel(q, k, v)
    print("out", o.shape, o.dtype, float(np.abs(o).max()))


# revision 6
# speedup vs baseline: 1.1142x; 1.1142x over previous
"""Dense dot-product attention (B=16, S=2048, D=128, fp32) on 8 TRN2 NeuronCores.

Sharding: data-parallel over batch — each of the 8 cores processes 2 full
batches independently (no collectives).

Per-core algorithm (per batch b, D=128, S=2048):

  - Load Q, K naturally ([s, d] tiles) as fp32; BITCAST to float32r (no
    cast pass — f32r matmul at moving>=256 is 1 cycle/row, same as fp16,
    and the DVE fp32->fp16 cast measured ~2ns/elem, 8us/batch, is avoided).
    V is cast to fp16 (PV matmul operand against fp16 P).
  - PE-transpose Q and K into [d, s] layout (QT, KT f32r).
  - Queries in 4 chunks of 512; k-tiles in 8 PAIRS per chunk. Per pair:
      S^T[k, q]  = 2 matmuls into one [128, 2, 512] PSUM pair tile (f32r)
      P^T[k, q]  = exp(S^T / sqrt(D)) — ONE ScalarE activation per pair
                   (1024 elem, halves the ~185ns/instr ScalarE bubble);
                   fp16 SBUF output. ScalarE is the pacing engine.
      O^T[d, q] += 2 matmuls(lhsT=V fp16, rhs=P^T slot)   (PSUM acc)
      Zacc      += P^T pair — elementwise, SPLIT between the otherwise-idle
                   Pool/GPSIMD engine (pure-fp16 ops; fp32-out measured
                   2.1ns/elem) and the DVE, per POOL_PAIRS/DVE_PAIRS.
                   This removes the ones-rowsum matmul from the PE (-25%
                   PE work vs computing Z on the tensor engine).
  - Chunk end (deferred into the next chunk's k-loop so no engine stalls):
      Z[q] = ones^T @ {Zacc_pool, Zacc_dve} slots — 4 accumulating matmuls
      into an [8, 512] PSUM tile (matmul cost scales with out FREE size, so
      8-partition replication makes the Z transpose ~16x cheaper);
      transpose Z columns ([8,128] -> [128,8], 8 moving rows each), DVE
      reciprocal; PE-transpose O^T back to [q, d] fp16; normalize during
      PSUM evacuation (tensor_scalar mul by 1/Z[q]); DMA out fp32.
"""

import math
import sys
from contextlib import ExitStack

try:
    import concourse.bass  # noqa: F401
except ImportError:
    for _p in ("/opt/trn_rl_repo", "/root/.axon_site/_ro/trn_rl_repo"):
        if _p not in sys.path:
            sys.path.insert(0, _p)

import numpy as np

import concourse.bass as bass
import concourse.mybir as mybir
import concourse.tile as tile
from concourse import bacc
from concourse.bass_utils import run_bass_kernel_spmd
from concourse.masks import make_identity

B, S, D = 16, 2048, 128
N_CORES = 8
B_LOC = B // N_CORES  # batches per core
P = 128
N_KT = S // P          # k tiles per batch (16)
N_KP = N_KT // 2       # k-tile pairs per chunk (8)
QCHUNK = 512           # queries per accumulation pass
N_QC = S // QCHUNK     # q chunks per batch (4)
NQT = QCHUNK // P      # output q tiles per chunk (4)
SOFTMAX_SCALE = 1.0 / math.sqrt(D)

# Which engine accumulates each k-pair's contribution to the softmax
# denominator (elementwise [128, 2, 512] adds). Tuned from trace: Pool
# (gpsimd) runs ~2.1ns/elem for fp32-out ops, DVE ~1ns/elem but is
# contested by evacuations/normalizes.
POOL_PAIRS = (0, 1, 2, 3, 4)
DVE_PAIRS = (5, 6, 7)

F32 = mybir.dt.float32
F32R = mybir.dt.float32r
F16 = mybir.dt.float16


def build_attention_nc() -> bass.Bass:
    nc = bacc.Bacc()
    q_in = nc.declare_dram_parameter("query", [B_LOC, S, D], F32, isOutput=False)
    k_in = nc.declare_dram_parameter("key", [B_LOC, S, D], F32, isOutput=False)
    v_in = nc.declare_dram_parameter("value", [B_LOC, S, D], F32, isOutput=False)
    o_out = nc.declare_dram_parameter("out", [B_LOC, S, D], F32, isOutput=True)

    with tile.TileContext(nc) as tc, ExitStack() as ctx:
        const = ctx.enter_context(tc.tile_pool(name="const", bufs=1))
        io = ctx.enter_context(tc.tile_pool(name="io", bufs=2))
        tr = ctx.enter_context(tc.tile_pool(name="tr", bufs=2))
        pexp = ctx.enter_context(tc.tile_pool(name="pexp", bufs=4))
        norm = ctx.enter_context(tc.tile_pool(name="norm", bufs=2))
        # PSUM: sc pairs 2x2 banks + outT 1 + z ring 2 + tp 1 = 8 banks
        ps_s = ctx.enter_context(tc.tile_pool(name="ps_s", bufs=2, space="PSUM"))
        ps_acc = ctx.enter_context(tc.tile_pool(name="ps_acc", bufs=1, space="PSUM"))

        identity = const.tile([P, P], F32)
        make_identity(nc, identity)
        identity_r = const.tile([P, P], F32R)
        nc.vector.tensor_copy(identity_r[:], identity[:])
        identity_h = const.tile([P, P], F16)
        nc.vector.tensor_copy(identity_h[:], identity[:])
        ones8_f = const.tile([P, 8], F32)
        nc.gpsimd.memset(ones8_f[:], 1.0)
        ones8_r = const.tile([P, 8], F32R)
        nc.vector.tensor_copy(ones8_r[:], ones8_f[:])
        ones8_h = const.tile([P, 8], F16)
        nc.vector.tensor_copy(ones8_h[:], ones8_f[:])

        pending_z = None
        pending_fin = None

        # ---- per-batch input prep, split into pipelinable steps ----
        def emit_v_half(v_nat, v_f16, b, h):
            sl = slice(h * (N_KT // 2), (h + 1) * (N_KT // 2))
            nc.sync.dma_start(
                v_nat[:, sl, :],
                v_in[b, h * (S // 2) : (h + 1) * (S // 2), :].rearrange(
                    "(t p) d -> p t d", p=P
                ),
            )
            nc.vector.tensor_copy(v_f16[:, sl, :], v_nat[:, sl, :])

        def emit_qk_load(src_in, b, j4, tagp):
            nat = io.tile(
                [P, 4, D], F32, tag="qknat", name=f"nat_{tagp}_{b}_{j4}", bufs=8
            )
            nc.sync.dma_start(
                nat[:],
                src_in[b, j4 * 4 * P : (j4 + 1) * 4 * P, :].rearrange(
                    "(t p) d -> p t d", p=P
                ),
            )
            return nat

        def emit_qk_transp(nat, b, j4, dst):
            # transpose the raw fp32 (2 cyc/row); the f32r ROUNDING happens
            # inside the PSUM->SBUF evacuation copy we need anyway, so no
            # separate cast pass ever touches the DVE
            pst = ps_s.tile([P, 4, P], F32, tag="tp", name=f"pst_{b}_{j4}", bufs=1)
            for jj in range(4):
                nc.tensor.transpose(pst[:, jj, :], nat[:, jj, :], identity[:])
            nc.vector.tensor_copy(
                dst[:, j4 * 4 * P : (j4 + 1) * 4 * P], pst[:]
            )

        def make_prep_steps(b):
            """Returns (qt, kt, v_f16, steps, deferred)."""
            qt = tr.tile([P, S], F32R, tag="qt", name=f"qt_{b}")
            kt = tr.tile([P, S], F32R, tag="kt", name=f"kt_{b}")
            v_nat = io.tile([P, N_KT, D], F32, tag="vnat", name=f"vnat_{b}")
            v_f16 = io.tile([P, N_KT, D], F16, tag="vf16", name=f"vf16_{b}")

            def qk_split(src_in, j4, dst, tagp):
                box = {}

                def load():
                    box["nat"] = emit_qk_load(src_in, b, j4, tagp)

                def transp():
                    emit_qk_transp(box["nat"], b, j4, dst)

                return load, transp

            kp_ = [qk_split(k_in, j4, kt, "k") for j4 in range(N_KT // 4)]
            qp_ = [qk_split(q_in, j4, qt, "q") for j4 in range(N_KT // 4)]

            if b == 0:
                steps = [
                    kp_[0][0], qp_[0][0],
                    kp_[0][1], qp_[0][1],
                    lambda: emit_v_half(v_nat, v_f16, b, 0),
                    kp_[1][0], kp_[2][0], kp_[3][0],
                    lambda: emit_v_half(v_nat, v_f16, b, 1),
                    qp_[1][0], qp_[2][0], qp_[3][0],
                ]
                deferred = [
                    kp_[1][1], kp_[2][1], kp_[3][1],
                    qp_[1][1], qp_[2][1], qp_[3][1],
                ]
            else:
                steps = []
                deferred = [
                    kp_[0][0], kp_[1][0], kp_[2][0], kp_[3][0],
                    qp_[0][0],
                    lambda: emit_v_half(v_nat, v_f16, b, 0),
                    lambda: emit_v_half(v_nat, v_f16, b, 1),
                    kp_[0][1], kp_[1][1], kp_[2][1], kp_[3][1],
                    qp_[0][1],
                    qp_[1][0], qp_[2][0], qp_[3][0],
                    qp_[1][1], qp_[2][1], qp_[3][1],
                ]
            return qt, kt, v_f16, steps, deferred

        prep = {0: make_prep_steps(0)}
        deferred_steps: list = []

        for b in range(B_LOC):
            qt, kt, v_f16, steps, deferred = prep[b]
            for st in steps:
                st()
            deferred_steps.extend(deferred)
            steps.clear()

            if b + 1 < B_LOC:
                prep[b + 1] = make_prep_steps(b + 1)
                deferred_steps.extend(prep[b + 1][3])
                deferred_steps.extend(prep[b + 1][4])
                prep[b + 1][3].clear()
                prep[b + 1][4].clear()

            def emit_sc_pair(q_lo, kp):
                sc = ps_s.tile(
                    [P, 2, QCHUNK], F32, tag=f"sc{kp % 2}", name=f"sc_{kp}", bufs=1
                )
                for i in range(2):
                    nc.tensor.matmul(
                        sc[:, i, :],
                        kt[:, (2 * kp + i) * P : (2 * kp + i + 1) * P],
                        qt[:, q_lo : q_lo + QCHUNK],
                        start=True,
                        stop=True,
                    )
                return sc

            def emit_epilogue_z(Zacc_p, Zacc_d, z_sb):
                # Z[q] row-sums: 4 accumulating matmuls over both partial
                # Zaccs; [8, 512] PSUM out
                z_ps = ps_s.tile([8, QCHUNK], F32, tag="z", name="z_ps", bufs=2)
                nc.tensor.matmul(
                    z_ps[:], ones8_h[:], Zacc_p[:, 0, :], start=True, stop=False
                )
                nc.tensor.matmul(
                    z_ps[:], ones8_h[:], Zacc_p[:, 1, :], start=False, stop=False
                )
                nc.tensor.matmul(
                    z_ps[:], ones8_r[:], Zacc_d[:, 0, :], start=False, stop=False
                )
                nc.tensor.matmul(
                    z_ps[:], ones8_r[:], Zacc_d[:, 1, :], start=False, stop=True
                )
                nc.vector.tensor_copy(z_sb[:], z_ps[:])

            def emit_epilogue_fin(b, q_lo, o_un, z_sb):
                # transpose Z columns: [8, 128] -> [128, 8] (8 moving rows)
                zt_ps = ps_s.tile([P, NQT, 8], F32, tag="z", name="zt_ps", bufs=2)
                for j in range(NQT):
                    nc.tensor.transpose(
                        zt_ps[:, j, :],
                        z_sb[:8, j * P : (j + 1) * P],
                        identity[:8, :8],
                    )
                zr = norm.tile([P, NQT], F32, tag="zr")
                nc.vector.reciprocal(zr[:], zt_ps[:, :, 0])

                pst = ps_s.tile([P, NQT, D], F16, tag="tp", name="pst_o", bufs=1)
                for j in range(NQT):
                    nc.tensor.transpose(
                        pst[:, j, :], o_un[:, j * P : (j + 1) * P], identity_h[:]
                    )
                out_sb = norm.tile([P, NQT, D], F32, tag="osb")
                for j in range(NQT):
                    nc.vector.tensor_scalar_mul(
                        out_sb[:, j, :], pst[:, j, :], zr[:, j : j + 1]
                    )
                nc.sync.dma_start(
                    o_out[b, q_lo : q_lo + QCHUNK, :].rearrange(
                        "(t p) d -> p t d", p=P
                    ),
                    out_sb[:],
                )

            for qc in range(N_QC):
                q_lo = qc * QCHUNK
                outT = ps_acc.tile([P, QCHUNK], F32, tag="outT", name="outT")
                Zacc_p = tr.tile(
                    [P, 2, QCHUNK], F16, tag="zacc_p", name=f"zacc_p_{b}_{qc}"
                )
                Zacc_d = tr.tile(
                    [P, 2, QCHUNK], F32R, tag="zacc_d", name=f"zacc_d_{b}_{qc}"
                )

                sc = emit_sc_pair(q_lo, 0)
                pool_first = dve_first = True
                for kp in range(N_KP):
                    pt = pexp.tile(
                        [P, 2, QCHUNK], F16, tag="pt", name=f"pt_{kp}", bufs=4
                    )
                    nc.scalar.activation(
                        pt[:],
                        sc[:],
                        mybir.ActivationFunctionType.Exp,
                        scale=SOFTMAX_SCALE,
                    )
                    # deferred prep must be emitted BEFORE the next score
                    # pair: Tile deps are emission-ordered, and sc(kp+1) may
                    # read kt columns written by a deferred transpose
                    if kp == 1 and pending_z is not None:
                        pending_z()
                        pending_z = None
                    elif kp == 2 and pending_fin is not None:
                        pending_fin()
                        pending_fin = None
                    elif deferred_steps and kp >= 1:
                        deferred_steps.pop(0)()
                    if kp + 1 < N_KP:
                        sc = emit_sc_pair(q_lo, kp + 1)
                    for i in range(2):
                        nc.tensor.matmul(
                            outT[:],
                            v_f16[:, 2 * kp + i, :],
                            pt[:, i, :],
                            start=(kp == 0 and i == 0),
                            stop=(kp == N_KP - 1 and i == 1),
                        )
                    # softmax denominator: elementwise accumulate, split
                    # across the idle Pool engine and the DVE
                    if kp in POOL_PAIRS:
                        if pool_first:
                            nc.gpsimd.tensor_copy(Zacc_p[:], pt[:])
                            pool_first = False
                        else:
                            nc.gpsimd.tensor_tensor(
                                Zacc_p[:], Zacc_p[:], pt[:], mybir.AluOpType.add
                            )
                    else:
                        if dve_first:
                            nc.vector.tensor_copy(Zacc_d[:], pt[:])
                            dve_first = False
                        else:
                            nc.vector.tensor_tensor(
                                Zacc_d[:], Zacc_d[:], pt[:], mybir.AluOpType.add
                            )

                # evacuate accumulator (frees the PSUM bank for next chunk)
                o_un = norm.tile([P, QCHUNK], F16, tag="o_un")
                nc.vector.tensor_copy(o_un[:], outT[:])
                z_sb = norm.tile([8, QCHUNK], F32, tag="z_sb")

                pending_z = (
                    lambda Zacc_p=Zacc_p, Zacc_d=Zacc_d, z_sb=z_sb: emit_epilogue_z(
                        Zacc_p, Zacc_d, z_sb
                    )
                )
                pending_fin = (
                    lambda b=b, q_lo=q_lo, o_un=o_un, z_sb=z_sb: emit_epilogue_fin(
                        b, q_lo, o_un, z_sb
                    )
                )

        if pending_z is not None:
            pending_z()
        if pending_fin is not None:
            pending_fin()
        while deferred_steps:
            deferred_steps.pop(0)()

    nc.compile()
    return nc


_NC_CACHE: bass.Bass | None = None


def _get_nc() -> bass.Bass:
    global _NC_CACHE
    if _NC_CACHE is None:
        _NC_CACHE = build_attention_nc()
    return _NC_CACHE


def kernel(query: np.ndarray, key: np.ndarray, value: np.ndarray) -> np.ndarray:
    query = np.ascontiguousarray(np.asarray(query, dtype=np.float32))
    key = np.ascontiguousarray(np.asarray(key, dtype=np.float32))
    value = np.ascontiguousarray(np.asarray(value, dtype=np.float32))
    assert query.shape == (B, S, D), query.shape

    nc = _get_nc()
    core_ids = list(range(N_CORES))
    in_maps = [
        {
            "query": query[i * B_LOC : (i + 1) * B_LOC],
            "key": key[i * B_LOC : (i + 1) * B_LOC],
            "value": value[i * B_LOC : (i + 1) * B_LOC],
        }
        for i in range(N_CORES)
    ]
    res = run_bass_kernel_spmd(nc, in_maps, core_ids)
    out = np.concatenate([res.results[i]["out"] for i in range(N_CORES)], axis=0)
    return out


if __name__ == "__main__":
    rng = np.random.default_rng(0)
    q = rng.standard_normal((B, S, D)).astype(np.float32)
    k = rng.standard_normal((B, S, D)).astype(np.float32)
    v = rng.standard_normal((B, S, D)).astype(np.float32)
    o = kernel(q, k, v)
    print("out", o.shape, o.dtype, float(np.abs(o).max()))
